# revision 69
# baseline (speedup 1.0000x reference)
"""AutoCorrelation kernel for Trainium2, 8 NeuronCores.

Math per (b, h) pair with X = x[b, :, h*64:(h+1)*64]  [T=2048, hd=64]:
  Xc = X - mean_T(X)
  S  = Xc @ Xc.T                  (symmetric!)
  P  = softmax(S, axis=-1)
  out = P @ X

Implementation exploits symmetry of E = exp(S - 64):
  out[t] = (sum_T' E[t,T'] X[T']) / (sum_T' E[t,T'])
and E == E.T, so the row-blocks of E computed with t on partitions can be
used directly as the *streaming* operand of the PV matmul (lhsT = [X | 1]),
which also yields the softmax denominator L in output row 64. No transposes
of the attention matrix are ever needed.

Engine plan per core (8 independent (b,h) pairs, data parallel across 8
cores): S-matmuls run 2x via PE row-tiling (K=64 on tiles T0/T8); exp is
split between ScalarE (table exp) and VectorE (Schraudolph bf16 bit-trick
with saturating f32->u16 convert); the PV matmuls for the previous pair are
spread between S blocks so ScalarE never starves; prep (DMA, paired
transposes, centering) for the next pair is interleaved too.
"""

import numpy as np

NCORES = 8
B, T, D, H = 4, 2048, 1024, 16
HD = D // H            # 64
PAIRS = B * H          # 64
PPC = PAIRS // NCORES  # 8 pairs per core
KT = T // 128          # 16 row-blocks of 128

# exp split between ScalarE (table exp, ~(N+352)/1.2 ns) and VectorE
# (Schraudolph bf16 bit-trick, f32 PSUM in -> 1x, ~(N+130)/0.96 ns).
# Each S block yields 4 single-bank [128,512] PSUM units; units j=(0,1)
# and (2,3) come from concurrent row-tiled matmul pairs, so their exp
# engines MUST alternate (s,v) on every adjacent slot pair -> both banks
# free together and ALL next-block matmul pairs stay concurrent. The
# forced 1:1 exp split is rebalanced by moving the copy-aux (vb/xb/sbt)
# to ScalarE.
UNIT_ENGINE = (("s", "v", "s", "v"), ("s", "v", "s", "v"))
SCHRAUD_A = 128.0 / float(np.log(2.0))               # 184.6649...
SCHRAUD_B = 127.0 * 128.0 - 5.25 - 64.0 * SCHRAUD_A  # bf16 bits bias, folds exp(-64)

_CACHE = {}


def _build_nc():
    import concourse.bass as bass  # noqa: F401
    import concourse.tile as tile
    from concourse import bacc, mybir
    from concourse.masks import make_identity

    f32 = mybir.dt.float32
    f16 = mybir.dt.float16
    bf16 = mybir.dt.bfloat16
    u16 = mybir.dt.uint16
    ADD = mybir.AluOpType.add
    MULT = mybir.AluOpType.mult
    EXP = mybir.ActivationFunctionType.Exp

    nc = bacc.Bacc(None)
    x_ext = nc.declare_dram_parameter("x", [PPC, T, HD], f32, isOutput=False)
    o_ext = nc.declare_dram_parameter("out", [PPC, T, HD], f32, isOutput=True)

    x_t = x_ext.ap().rearrange("p (ko pp) d -> p pp ko d", pp=128)
    o_t = o_ext.ap().rearrange("p (ko pp) d -> p pp ko d", pp=128)

    with tile.TileContext(nc) as tc:
        with (
            tc.tile_pool(name="const", bufs=1) as constp,
            tc.tile_pool(name="xst", bufs=2) as xstp,
            tc.tile_pool(name="stage", bufs=2) as stagep,
            tc.tile_pool(name="xct", bufs=2) as xctp,
            tc.tile_pool(name="vb", bufs=2) as vbp,
            tc.tile_pool(name="eb", bufs=2) as ebp,
            tc.tile_pool(name="osb", bufs=2) as osbp,
            tc.tile_pool(name="sbt", bufs=2) as sbtp,
            tc.tile_pool(name="small", bufs=4) as smallp,
            tc.tile_pool(name="psS", bufs=6, space="PSUM") as psSp,
            tc.tile_pool(name="psM", bufs=2, space="PSUM") as psMp,
        ):
            neg64 = constp.tile([128, 1], f32)
            nc.vector.memset(neg64, -64.0)
            ident = constp.tile([128, 128], bf16)
            make_identity(nc, ident)
            identf = constp.tile([HD + 1, HD + 1], f32)
            make_identity(nc, identf)

            state = {}
            xst_tiles = {}

            def emit_dma_in(p):
                # split across two DMA queues: the 256B-element gather is
                # descriptor-rate-bound (~5.7us on one queue)
                xst = xstp.tile([128, KT, HD], f32, tag="xst")
                nc.gpsimd.dma_start(xst[:, 0:8], x_t[p][:, 0:8])
                nc.sync.dma_start(xst[:, 8:16], x_t[p][:, 8:16])
                xst_tiles[p] = xst

            def emit_prep_a(p):
                # V (with ones column) + packed bf16 copy for DMA-transposes
                xst = xst_tiles[p]
                vb = vbp.tile([128, KT, HD + 1], bf16, tag="vb")
                nc.vector.memset(vb[:, :, HD : HD + 1], 1.0)
                nc.scalar.activation(
                    vb[:, :, 0:HD], xst,
                    mybir.ActivationFunctionType.Copy,
                )
                xb = stagep.tile([128, KT * HD], bf16, tag="xb")
                nc.scalar.activation(
                    xb.rearrange("p (k d) -> p k d", d=HD), xst,
                    mybir.ActivationFunctionType.Copy,
                )
                xct = xctp.tile([128, T], bf16, tag="xct")
                E = ebp.tile([128, KT, T], bf16, tag="eb")
                state[p] = {"E": E, "vb": vb, "xct": xct, "xb": xb}
                if p > 0:
                    state[p]["stage"] = stagep.tile(
                        [128, 2 * 512], bf16, tag="stage", name="stage"
                    )
                xst_tiles.pop(p)

            def emit_prep_b(p):
                # XT: out[p, b, f] = in[f, b*128+p] (partitions 0:64 get
                # even k-tile's d, 64:128 odd k-tile's d). Steady state uses
                # the HW DMA-transpose; pair 0 uses PE transposes instead
                # (PE idle at startup, and the DMA xbar has ~9us latency
                # that would sit on the startup critical path).
                xb = state[p]["xb"]
                if p > 0:
                    nc.sync.dma_start_transpose(
                        state[p]["stage"].rearrange("p (q f) -> p q f", f=128),
                        xb,
                    )
                    return
                xbv = xb.rearrange("p (k d) -> p k d", d=HD)
                psT = [
                    psMp.tile([HD, 1024], bf16, tag="mix", name=f"psT{h}")
                    for h in range(2)
                ]
                for k in range(KT):
                    nc.tensor.transpose(
                        psT[k % 2][:, (k // 2) * 128 : (k // 2 + 1) * 128],
                        xbv[:, k, :], ident,
                    )
                state[p]["psT"] = psT

            def emit_prep_bc0():
                # pair-0-only fast path: mean + centering straight from the
                # PE-transposed PSUM tiles into xct's upper half (engines
                # cannot cross partitions), then one big-descriptor DMA
                # duplicates partitions 0:64 -> 64:128.
                psT = state[0].pop("psT")
                xct = state[0]["xct"]
                xg = xct.rearrange("p (k f) -> p k f", f=128)
                part0 = smallp.tile([HD, 1], f32, tag="part")
                nc.vector.tensor_reduce(
                    part0, psT[0], mybir.AxisListType.X, ADD
                )
                scr = sbtp.tile([HD, 1024], bf16, tag="scr0", name="scr0")
                part1 = smallp.tile([HD, 1], f32, tag="swap", name="part1")
                nc.scalar.activation(
                    scr, psT[1], mybir.ActivationFunctionType.Identity,
                    accum_out=part1,
                )
                # the rest of the chain avoids the vector queue: the
                # scheduler likes hoisting pair-1 prep ahead of these ops,
                # which would head-of-line block them for ~7us
                mu64 = smallp.tile([HD, 1], f32, tag="mufull", name="mu64")
                nc.gpsimd.tensor_tensor(mu64, part0, part1, ADD)
                nc.gpsimd.tensor_scalar_mul(mu64, mu64, -1.0 / T)
                for h in range(2):
                    nc.scalar.activation(
                        xg[0:HD, h : KT : 2, :],
                        psT[h].rearrange("p (k f) -> p k f", f=128),
                        mybir.ActivationFunctionType.Identity,
                        bias=mu64,
                    )
                # centered upper half -> duplicate to partitions 64:128
                nc.sync.dma_start(xct[HD:128, :], xct[0:HD, :])

            def emit_prep_b2(p):
                # mean over T from the transposed stage (free-axis reduce).
                # Partitions 0:64 and 64:128 hold the even/odd k-tile
                # partial sums of the same d; a half-swap stream_shuffle +
                # add gives the full sum in BOTH halves at once (symmetric),
                # so no cross-partition DMA or broadcast is needed.
                stage = state[p]["stage"]
                part = smallp.tile([128, 1], f32, tag="part")
                nc.vector.tensor_reduce(
                    part, stage.rearrange("p (q f) -> p q f", f=128),
                    mybir.AxisListType.XY, ADD,
                )
                swap = smallp.tile([128, 1], f32, tag="swap")
                nc.vector.stream_shuffle(
                    swap, part, [(g + 16) % 32 for g in range(32)]
                )
                mufull = smallp.tile([128, 1], f32, tag="mufull")
                nc.vector.tensor_tensor(part, part, swap, ADD)
                nc.vector.tensor_scalar_mul(mufull, part, -1.0 / T)
                if p == 0:
                    # startup: keep the centering off the vector queue so
                    # pair-1 prep scheduled ahead of it cannot stall it
                    nc.scalar.activation(
                        stage, stage,
                        mybir.ActivationFunctionType.Identity, bias=mufull,
                    )
                else:
                    nc.vector.tensor_scalar(stage, stage, mufull, None, ADD)
                state[p]["mufull"] = mufull

            def emit_prep_c(p):
                # shuffle stage -> xct (both halves get all 16 k-tiles);
                # pair 0's shuffles are on the startup critical path, so
                # half go out on the sync queue in parallel
                stage = state[p]["stage"]
                xct = state[p]["xct"]
                sg = stage.rearrange("p (q f) -> p q f", f=128)
                xg = xct.rearrange("p (k f) -> p k f", f=128)
                eng = nc.sync if p == 0 else nc.gpsimd
                nc.gpsimd.dma_start(xg[0:HD, 0:KT:2, :], sg[0:HD])
                eng.dma_start(xg[0:HD, 1:KT:2, :], sg[HD:128])
                nc.gpsimd.dma_start(xg[HD:128, 0:KT:2, :], sg[0:HD])
                eng.dma_start(xg[HD:128, 1:KT:2, :], sg[HD:128])

            def emit_s_exp(p, m):
                E = state[p]["E"]
                xct = state[p]["xct"]
                ms = slice(m * 128, (m + 1) * 128)
                # four [128,512] single-bank units; (0,1) and (2,3) are
                # concurrent row-tiled pairs: j even = rows 0:63 (T0, cols
                # 0:1024), j odd = rows 64:127 (T8, cols 1024:2048).
                ps = [
                    psSp.tile([128, 512], f32, tag="psS", name=f"psS{j}")
                    for j in range(4)
                ]
                for n in range(2):
                    nc.tensor.matmul(
                        ps[2 * n],
                        lhsT=xct[0:HD, ms],
                        rhs=xct[0:HD, n * 512 : (n + 1) * 512],
                        start=True, stop=True, tile_position=(0, 0),
                    )
                    nc.tensor.matmul(
                        ps[2 * n + 1],
                        lhsT=xct[HD:128, ms],
                        rhs=xct[HD:128, 1024 + n * 512 : 1024 + (n + 1) * 512],
                        start=True, stop=True, tile_position=(64, 0),
                    )
                cols = (0, 1024, 512, 1536)
                for j in range(4):
                    eview = E[:, m, cols[j] : cols[j] + 512]
                    if UNIT_ENGINE[m % 2][j] == "v":
                        # Schraudolph in bf16 bit-space; f32->u16 convert
                        # saturates negatives to 0 (== exp underflow).
                        nc.vector.tensor_scalar(
                            eview.bitcast(u16), ps[j], SCHRAUD_A, SCHRAUD_B,
                            MULT, ADD,
                        )
                    else:
                        nc.scalar.activation(
                            eview, ps[j], EXP, bias=neg64, scale=1.0
                        )

            pv_live = {}

            def emit_pv_part(q, c, part):
                # 8 of the 16 accumulating PV matmuls for chunk c of pair q
                E, vb = state[q]["E"], state[q]["vb"]
                cs = slice(c * 512, (c + 1) * 512)
                if part == 0:
                    pv_live[(q, c)] = psMp.tile(
                        [HD + 1, 512], f32, tag="mix", name="pspv"
                    )
                pspv = pv_live[(q, c)]
                for kk in range(8):
                    k = part * 8 + kk
                    nc.tensor.matmul(
                        pspv,
                        lhsT=vb[:, k, :],
                        rhs=E[:, k, cs],
                        start=(k == 0), stop=(k == KT - 1),
                        skip_group_check=True,
                    )

            sbts = {}

            def emit_pv_tail(q, c):
                # PSUM -> bf16 SBUF copy per chunk; frees the PV PSUM slot.
                pspv = pv_live.pop((q, c))
                if q == PPC - 1:
                    # last pair: per-chunk PE transposes (PE idles in the
                    # final phase and the xbar's ~5us dispatch latency would
                    # serialize the drain).
                    sbt = sbtp.tile(
                        [HD + 1, 512], f32, tag="sbtL", name="sbtL"
                    )
                    nc.vector.tensor_copy(sbt, pspv)
                    ps2 = psMp.tile(
                        [128, 4, HD + 1], f32, tag="mix", name="ps2L"
                    )
                    for j in range(4):
                        nc.tensor.transpose(
                            ps2[:, j, :],
                            sbt[:, j * 128 : (j + 1) * 128],
                            identf,
                        )
                    lrec = smallp.tile([128, 4], f32, tag="lrecL", name="lrecL")
                    nc.vector.reciprocal(lrec, ps2[:, :, HD])
                    osbc = osbp.tile([128, 4, HD], f32, tag="osbL", name="osbL")
                    nc.vector.tensor_tensor(
                        osbc, ps2[:, :, 0:HD],
                        lrec[:, :, None].to_broadcast([128, 4, HD]), MULT,
                    )
                    eng = nc.gpsimd if c % 2 == 0 else nc.sync
                    eng.dma_start(o_t[q][:, c * 4 : (c + 1) * 4, :], osbc)
                    return
                if c == 0:
                    sbts[q] = sbtp.tile([80, T], bf16, tag="sbt", name="sbt")
                nc.scalar.activation(
                    sbts[q][0 : HD + 1, c * 512 : (c + 1) * 512], pspv,
                    mybir.ActivationFunctionType.Copy,
                )


            out_live = {}

            def emit_pair_out_a(q):
                # ONE whole-pair HW DMA-transpose (xbar, 16x128 tiles; 80 =
                # padded partition count) gives the [t, d] layout without
                # touching the Tensor engine. Emitted an iteration after the
                # last sbt copy so its wait never head-of-line blocks the
                # next pair's prep transpose on the sync queue. Rows 65:80
                # of sbt are never written and land at f=65:80 of ps2t,
                # outside every consumed slice.
                sbt = sbts.pop(q)
                ps2t = sbtp.tile([128, KT, 80], bf16, tag="ps2t")
                nc.sync.dma_start_transpose(ps2t, sbt)
                out_live[q] = ps2t

            def emit_pair_out_b(q):
                # emitted ~5us after the dispatch so the xbar's transfer
                # latency never stalls the vector queue head
                ps2t = out_live.pop(q)
                lrec = smallp.tile([128, KT], f32, tag="lrec")
                nc.vector.reciprocal(lrec, ps2t[:, :, HD])
                osb = osbp.tile([128, KT, HD], f32, tag="osb")
                nc.vector.tensor_tensor(
                    osb, ps2t[:, :, 0:HD],
                    lrec[:, :, None].to_broadcast([128, KT, HD]), MULT,
                )
                nc.gpsimd.dma_start(o_t[q][:, 0:8], osb[:, 0:8])
                nc.sync.dma_start(o_t[q][:, 8:16], osb[:, 8:16])

            emit_dma_in(0)
            emit_prep_a(0)
            emit_prep_b(0)
            emit_prep_bc0()
            emit_dma_in(1)
            # HAM warm-up: transpose-mode matmuls don't count as PE-busy,
            # so without these the first real S matmuls run at 1.2 GHz.
            # 18 allocations keep the psS slot parity (18 % 6 == 0).
            xb0 = state[0]["xb"]
            for w in range(18):
                psW = psSp.tile([128, 512], f32, tag="psS", name="psW")
                nc.tensor.matmul(
                    psW, lhsT=ident, rhs=xb0[:, 0:512],
                    start=True, stop=True,
                )
            for it in range(PPC + 1):
                for m in range(KT):
                    if it < PPC:
                        emit_s_exp(it, m)
                    if it > 0 and m % 2 == 1:
                        emit_pv_part(it - 1, m // 4, (m % 4) // 2)
                        if m % 4 == 3 and it < PPC:
                            emit_pv_tail(it - 1, m // 4)
                        # last pair: delay each tail 2 slots so its PE
                        # transposes never head-of-line block the next
                        # PV group behind the sbt copy
                        if it == PPC and m % 4 == 1 and m > 4:
                            emit_pv_tail(it - 1, (m - 5) // 4)
                    if m == 5 and it >= 2 and it - 2 < PPC - 1:
                        emit_pair_out_a(it - 2)
                    if m == 1 and it >= 3 and it - 3 < PPC - 1:
                        emit_pair_out_b(it - 3)
                    if it + 1 < PPC:
                        if m == 2:
                            emit_prep_a(it + 1)
                        elif m == 3:
                            emit_prep_b(it + 1)
                        elif m == 6:
                            emit_prep_b2(it + 1)
                        elif m == 7:
                            emit_prep_c(it + 1)
                        elif m == 9 and it + 2 < PPC:
                            emit_dma_in(it + 2)
                if it > 0:
                    state.pop(it - 1)
            emit_pv_tail(PPC - 1, 3)
            emit_pair_out_b(PPC - 2)
    nc.compile()
    return nc


def _get_nc():
    if "nc" not in _CACHE:
        _CACHE["nc"] = _build_nc()
    return _CACHE["nc"]


def kernel(x: np.ndarray) -> np.ndarray:
    from concourse.bass_utils import run_bass_kernel_spmd

    nc = _get_nc()
    x = np.asarray(x, dtype=np.float32)
    xh = (
        x.reshape(B, T, H, HD).transpose(0, 2, 1, 3).reshape(PAIRS, T, HD)
    )
    in_maps = [
        {"x": np.ascontiguousarray(xh[i * PPC : (i + 1) * PPC])}
        for i in range(NCORES)
    ]
    for _attempt in range(3):
        res = run_bass_kernel_spmd(nc, in_maps, core_ids=list(range(NCORES)))
        outs = np.concatenate(
            [np.asarray(res.results[i]["out"]) for i in range(NCORES)], axis=0
        )
        if np.isfinite(outs).all():
            break
    return (
        outs.reshape(B, H, T, HD).transpose(0, 2, 1, 3).reshape(B, T, D)
    ).astype(np.float32)



# revision 70
# speedup vs baseline: 1.1362x; 1.1362x over previous
"""AutoCorrelation kernel for Trainium2, 8 NeuronCores.

Math per (b, h) pair with X = x[b, :, h*64:(h+1)*64]  [T=2048, hd=64]:
  Xc = X - mean_T(X)
  S  = Xc @ Xc.T                  (symmetric!)
  P  = softmax(S, axis=-1)
  out = P @ X

Implementation exploits symmetry of E = exp(S - 64):
  out[t] = (sum_T' E[t,T'] X[T']) / (sum_T' E[t,T'])
and E == E.T, so the row-blocks of E computed with t on partitions can be
used directly as the *streaming* operand of the PV matmul (lhsT = [X | 1]),
which also yields the softmax denominator L in output row 64. No transposes
of the attention matrix are ever needed.

Engine plan per core (8 independent (b,h) pairs, data parallel across 8
cores): S-matmuls run 2x via PE row-tiling (K=64 on tiles T0/T8); exp is
split between ScalarE (table exp) and VectorE (Schraudolph bf16 bit-trick
with saturating f32->u16 convert); the PV matmuls for the previous pair are
spread between S blocks so ScalarE never starves; prep (DMA, paired
transposes, centering) for the next pair is interleaved too.
"""

import numpy as np

NCORES = 8
B, T, D, H = 4, 2048, 1024, 16
HD = D // H            # 64
PAIRS = B * H          # 64
PPC = PAIRS // NCORES  # 8 pairs per core
KT = T // 128          # 16 row-blocks of 128

# exp split between ScalarE (table exp, ~(N+352)/1.2 ns) and VectorE
# (Schraudolph bf16 bit-trick, f32 PSUM in -> 1x, ~(N+130)/0.96 ns).
# Each S block yields 4 single-bank [128,512] PSUM units; units j=(0,1)
# and (2,3) come from concurrent row-tiled matmul pairs, so their exp
# engines MUST alternate (s,v) on every adjacent slot pair -> both banks
# free together and ALL next-block matmul pairs stay concurrent. The
# forced 1:1 exp split is rebalanced by moving the copy-aux (vb/xb/sbt)
# to ScalarE.
UNIT_ENGINE = (("s", "v", "s", "v"), ("s", "v", "s", "v"))
SCHRAUD_A = 128.0 / float(np.log(2.0))               # 184.6649...
SCHRAUD_B = 127.0 * 128.0 - 5.25 - 64.0 * SCHRAUD_A  # bf16 bits bias, folds exp(-64)

_CACHE = {}


def _build_nc():
    import concourse.bass as bass  # noqa: F401
    import concourse.tile as tile
    from concourse import bacc, mybir
    from concourse.masks import make_identity

    f32 = mybir.dt.float32
    f16 = mybir.dt.float16
    bf16 = mybir.dt.bfloat16
    u16 = mybir.dt.uint16
    ADD = mybir.AluOpType.add
    MULT = mybir.AluOpType.mult
    EXP = mybir.ActivationFunctionType.Exp

    nc = bacc.Bacc(None)
    x_ext = nc.declare_dram_parameter("x", [PPC, T, HD], f32, isOutput=False)
    o_ext = nc.declare_dram_parameter("out", [PPC, T, HD], f32, isOutput=True)

    x_t = x_ext.ap().rearrange("p (ko pp) d -> p pp ko d", pp=128)
    o_t = o_ext.ap().rearrange("p (ko pp) d -> p pp ko d", pp=128)

    with tile.TileContext(nc) as tc:
        with (
            tc.tile_pool(name="const", bufs=1) as constp,
            tc.tile_pool(name="xst", bufs=2) as xstp,
            tc.tile_pool(name="stage", bufs=2) as stagep,
            tc.tile_pool(name="xct", bufs=2) as xctp,
            tc.tile_pool(name="vb", bufs=2) as vbp,
            tc.tile_pool(name="eb", bufs=2) as ebp,
            tc.tile_pool(name="osb", bufs=2) as osbp,
            tc.tile_pool(name="sbt", bufs=2) as sbtp,
            tc.tile_pool(name="small", bufs=4) as smallp,
            tc.tile_pool(name="psS", bufs=6, space="PSUM") as psSp,
            tc.tile_pool(name="psM", bufs=2, space="PSUM") as psMp,
        ):
            neg64 = constp.tile([128, 1], f32)
            nc.vector.memset(neg64, -64.0)
            ident = constp.tile([128, 128], bf16)
            make_identity(nc, ident)
            identf = constp.tile([HD + 1, HD + 1], f32)
            make_identity(nc, identf)

            state = {}
            xst_tiles = {}

            def emit_dma_in(p):
                # split across two DMA queues: the 256B-element gather is
                # descriptor-rate-bound (~5.7us on one queue)
                xst = xstp.tile([128, KT, HD], f32, tag="xst")
                nc.gpsimd.dma_start(xst[:, 0:8], x_t[p][:, 0:8])
                nc.sync.dma_start(xst[:, 8:16], x_t[p][:, 8:16])
                xst_tiles[p] = xst

            def emit_prep_a(p):
                # V (with ones column) + packed bf16 copy for DMA-transposes
                xst = xst_tiles[p]
                vb = vbp.tile([128, KT, HD + 1], bf16, tag="vb")
                nc.vector.memset(vb[:, :, HD : HD + 1], 1.0)
                nc.scalar.activation(
                    vb[:, :, 0:HD], xst,
                    mybir.ActivationFunctionType.Copy,
                )
                xb = stagep.tile([128, KT * HD], bf16, tag="xb")
                nc.scalar.activation(
                    xb.rearrange("p (k d) -> p k d", d=HD), xst,
                    mybir.ActivationFunctionType.Copy,
                )
                xct = xctp.tile([128, T], bf16, tag="xct")
                E = ebp.tile([128, KT, T], bf16, tag="eb")
                state[p] = {"E": E, "vb": vb, "xct": xct, "xb": xb}
                if p > 0:
                    state[p]["stage"] = stagep.tile(
                        [128, 2 * 512], bf16, tag="stage", name="stage"
                    )
                xst_tiles.pop(p)

            def emit_prep_b(p):
                # XT: out[p, b, f] = in[f, b*128+p] (partitions 0:64 get
                # even k-tile's d, 64:128 odd k-tile's d). Steady state uses
                # the HW DMA-transpose; pair 0 uses PE transposes instead
                # (PE idle at startup, and the DMA xbar has ~9us latency
                # that would sit on the startup critical path).
                xb = state[p]["xb"]
                if p > 0:
                    nc.sync.dma_start_transpose(
                        state[p]["stage"].rearrange("p (q f) -> p q f", f=128),
                        xb,
                    )
                    return
                xbv = xb.rearrange("p (k d) -> p k d", d=HD)
                psT = [
                    psMp.tile([HD, 1024], bf16, tag="mix", name=f"psT{h}")
                    for h in range(2)
                ]
                for k in range(KT):
                    nc.tensor.transpose(
                        psT[k % 2][:, (k // 2) * 128 : (k // 2 + 1) * 128],
                        xbv[:, k, :], ident,
                    )
                state[p]["psT"] = psT

            def emit_prep_bc0():
                # pair-0-only fast path: mean + centering straight from the
                # PE-transposed PSUM tiles into xct's upper half (engines
                # cannot cross partitions), then one big-descriptor DMA
                # duplicates partitions 0:64 -> 64:128.
                psT = state[0].pop("psT")
                xct = state[0]["xct"]
                xg = xct.rearrange("p (k f) -> p k f", f=128)
                part0 = smallp.tile([HD, 1], f32, tag="part")
                nc.vector.tensor_reduce(
                    part0, psT[0], mybir.AxisListType.X, ADD
                )
                scr = sbtp.tile([HD, 1024], bf16, tag="scr0", name="scr0")
                part1 = smallp.tile([HD, 1], f32, tag="swap", name="part1")
                nc.scalar.activation(
                    scr, psT[1], mybir.ActivationFunctionType.Identity,
                    accum_out=part1,
                )
                # the rest of the chain avoids the vector queue: the
                # scheduler likes hoisting pair-1 prep ahead of these ops,
                # which would head-of-line block them for ~7us
                mu64 = smallp.tile([HD, 1], f32, tag="mufull", name="mu64")
                nc.gpsimd.tensor_tensor(mu64, part0, part1, ADD)
                nc.gpsimd.tensor_scalar_mul(mu64, mu64, -1.0 / T)
                for h in range(2):
                    nc.scalar.activation(
                        xg[0:HD, h : KT : 2, :],
                        psT[h].rearrange("p (k f) -> p k f", f=128),
                        mybir.ActivationFunctionType.Identity,
                        bias=mu64,
                    )
                # centered upper half -> duplicate to partitions 64:128
                nc.sync.dma_start(xct[HD:128, :], xct[0:HD, :])

            def emit_prep_b2(p):
                # mean over T from the transposed stage (free-axis reduce).
                # Partitions 0:64 and 64:128 hold the even/odd k-tile
                # partial sums of the same d; a half-swap stream_shuffle +
                # add gives the full sum in BOTH halves at once (symmetric),
                # so no cross-partition DMA or broadcast is needed.
                stage = state[p]["stage"]
                part = smallp.tile([128, 1], f32, tag="part")
                nc.vector.tensor_reduce(
                    part, stage.rearrange("p (q f) -> p q f", f=128),
                    mybir.AxisListType.XY, ADD,
                )
                swap = smallp.tile([128, 1], f32, tag="swap")
                nc.vector.stream_shuffle(
                    swap, part, [(g + 16) % 32 for g in range(32)]
                )
                mufull = smallp.tile([128, 1], f32, tag="mufull")
                nc.vector.tensor_tensor(part, part, swap, ADD)
                nc.vector.tensor_scalar_mul(mufull, part, -1.0 / T)
                if p == 0:
                    # startup: keep the centering off the vector queue so
                    # pair-1 prep scheduled ahead of it cannot stall it
                    nc.scalar.activation(
                        stage, stage,
                        mybir.ActivationFunctionType.Identity, bias=mufull,
                    )
                else:
                    nc.vector.tensor_scalar(stage, stage, mufull, None, ADD)
                state[p]["mufull"] = mufull

            def emit_prep_c(p):
                # shuffle stage -> xct (both halves get all 16 k-tiles);
                # pair 0's shuffles are on the startup critical path, so
                # half go out on the sync queue in parallel
                stage = state[p]["stage"]
                xct = state[p]["xct"]
                sg = stage.rearrange("p (q f) -> p q f", f=128)
                xg = xct.rearrange("p (k f) -> p k f", f=128)
                eng = nc.sync if p == 0 else nc.gpsimd
                nc.gpsimd.dma_start(xg[0:HD, 0:KT:2, :], sg[0:HD])
                eng.dma_start(xg[0:HD, 1:KT:2, :], sg[HD:128])
                nc.gpsimd.dma_start(xg[HD:128, 0:KT:2, :], sg[0:HD])
                eng.dma_start(xg[HD:128, 1:KT:2, :], sg[HD:128])

            def emit_s_exp(p, m):
                E = state[p]["E"]
                xct = state[p]["xct"]
                ms = slice(m * 128, (m + 1) * 128)
                # four [128,512] single-bank units; (0,1) and (2,3) are
                # concurrent row-tiled pairs: j even = rows 0:63 (T0, cols
                # 0:1024), j odd = rows 64:127 (T8, cols 1024:2048).
                ps = [
                    psSp.tile([128, 512], f32, tag="psS", name=f"psS{j}")
                    for j in range(4)
                ]
                for n in range(2):
                    nc.tensor.matmul(
                        ps[2 * n],
                        lhsT=xct[0:HD, ms],
                        rhs=xct[0:HD, n * 512 : (n + 1) * 512],
                        start=True, stop=True, tile_position=(0, 0),
                    )
                    nc.tensor.matmul(
                        ps[2 * n + 1],
                        lhsT=xct[HD:128, ms],
                        rhs=xct[HD:128, 1024 + n * 512 : 1024 + (n + 1) * 512],
                        start=True, stop=True, tile_position=(64, 0),
                    )
                cols = (0, 1024, 512, 1536)
                for j in range(4):
                    eview = E[:, m, cols[j] : cols[j] + 512]
                    if UNIT_ENGINE[m % 2][j] == "v":
                        # Schraudolph in bf16 bit-space; f32->u16 convert
                        # saturates negatives to 0 (== exp underflow).
                        nc.vector.tensor_scalar(
                            eview.bitcast(u16), ps[j], SCHRAUD_A, SCHRAUD_B,
                            MULT, ADD,
                        )
                    else:
                        nc.scalar.activation(
                            eview, ps[j], EXP, bias=neg64, scale=1.0
                        )

            pv_live = {}

            def emit_pv_part(q, c, part):
                # 8 of the 16 accumulating PV matmuls for chunk c of pair q
                E, vb = state[q]["E"], state[q]["vb"]
                cs = slice(c * 512, (c + 1) * 512)
                if part == 0:
                    pv_live[(q, c)] = psMp.tile(
                        [HD + 1, 512], f32, tag="mix", name="pspv"
                    )
                pspv = pv_live[(q, c)]
                for kk in range(8):
                    k = part * 8 + kk
                    nc.tensor.matmul(
                        pspv,
                        lhsT=vb[:, k, :],
                        rhs=E[:, k, cs],
                        start=(k == 0), stop=(k == KT - 1),
                        skip_group_check=True,
                    )

            sbts = {}

            def emit_pv_tail(q, c):
                # PSUM -> bf16 SBUF copy per chunk; frees the PV PSUM slot.
                pspv = pv_live.pop((q, c))
                if q == PPC - 1:
                    # last pair: per-chunk PE transposes (PE idles in the
                    # final phase and the xbar's ~5us dispatch latency would
                    # serialize the drain).
                    sbt = sbtp.tile(
                        [HD + 1, 512], f32, tag="sbtL", name="sbtL"
                    )
                    nc.vector.tensor_copy(sbt, pspv)
                    ps2 = psMp.tile(
                        [128, 4, HD + 1], f32, tag="mix", name="ps2L"
                    )
                    for j in range(4):
                        nc.tensor.transpose(
                            ps2[:, j, :],
                            sbt[:, j * 128 : (j + 1) * 128],
                            identf,
                        )
                    lrec = smallp.tile([128, 4], f32, tag="lrecL", name="lrecL")
                    nc.vector.reciprocal(lrec, ps2[:, :, HD])
                    osbc = osbp.tile([128, 4, HD], f32, tag="osbL", name="osbL")
                    nc.vector.tensor_tensor(
                        osbc, ps2[:, :, 0:HD],
                        lrec[:, :, None].to_broadcast([128, 4, HD]), MULT,
                    )
                    eng = nc.gpsimd if c % 2 == 0 else nc.sync
                    eng.dma_start(o_t[q][:, c * 4 : (c + 1) * 4, :], osbc)
                    return
                if c == 0:
                    sbts[q] = sbtp.tile([80, T], bf16, tag="sbt", name="sbt")
                nc.scalar.activation(
                    sbts[q][0 : HD + 1, c * 512 : (c + 1) * 512], pspv,
                    mybir.ActivationFunctionType.Copy,
                )


            out_live = {}

            def emit_pair_out_a(q):
                # ONE whole-pair HW DMA-transpose (xbar, 16x128 tiles; 80 =
                # padded partition count) gives the [t, d] layout without
                # touching the Tensor engine. Emitted an iteration after the
                # last sbt copy so its wait never head-of-line blocks the
                # next pair's prep transpose on the sync queue. Rows 65:80
                # of sbt are never written and land at f=65:80 of ps2t,
                # outside every consumed slice.
                sbt = sbts.pop(q)
                ps2t = sbtp.tile([128, KT, 80], bf16, tag="ps2t")
                nc.sync.dma_start_transpose(ps2t, sbt)
                out_live[q] = ps2t

            def emit_pair_out_b(q):
                # emitted ~5us after the dispatch so the xbar's transfer
                # latency never stalls the vector queue head
                ps2t = out_live.pop(q)
                lrec = smallp.tile([128, KT], f32, tag="lrec")
                nc.vector.reciprocal(lrec, ps2t[:, :, HD])
                osb = osbp.tile([128, KT, HD], f32, tag="osb")
                nc.vector.tensor_tensor(
                    osb, ps2t[:, :, 0:HD],
                    lrec[:, :, None].to_broadcast([128, KT, HD]), MULT,
                )
                nc.gpsimd.dma_start(o_t[q][:, 0:8], osb[:, 0:8])
                nc.sync.dma_start(o_t[q][:, 8:16], osb[:, 8:16])

            emit_dma_in(0)
            emit_prep_a(0)
            emit_prep_b(0)
            emit_prep_bc0()
            emit_dma_in(1)

            for it in range(PPC + 1):
                for m in range(KT):
                    if it < PPC:
                        emit_s_exp(it, m)
                    if it > 0 and m % 2 == 1:
                        emit_pv_part(it - 1, m // 4, (m % 4) // 2)
                        if m % 4 == 3 and it < PPC:
                            emit_pv_tail(it - 1, m // 4)
                        # last pair: delay each tail 2 slots so its PE
                        # transposes never head-of-line block the next
                        # PV group behind the sbt copy
                        if it == PPC and m % 4 == 1 and m > 4:
                            emit_pv_tail(it - 1, (m - 5) // 4)
                    if m == 5 and it >= 2 and it - 2 < PPC - 1:
                        emit_pair_out_a(it - 2)
                    if m == 1 and it >= 3 and it - 3 < PPC - 1:
                        emit_pair_out_b(it - 3)
                    if it + 1 < PPC:
                        if m == 2:
                            emit_prep_a(it + 1)
                        elif m == 3:
                            emit_prep_b(it + 1)
                        elif m == 6:
                            emit_prep_b2(it + 1)
                        elif m == 7:
                            emit_prep_c(it + 1)
                        elif m == 9 and it + 2 < PPC:
                            emit_dma_in(it + 2)
                if it > 0:
                    state.pop(it - 1)
            emit_pv_tail(PPC - 1, 3)
            emit_pair_out_b(PPC - 2)
    nc.compile()
    return nc


def _get_nc():
    if "nc" not in _CACHE:
        _CACHE["nc"] = _build_nc()
    return _CACHE["nc"]


def kernel(x: np.ndarray) -> np.ndarray:
    from concourse.bass_utils import run_bass_kernel_spmd

    nc = _get_nc()
    x = np.asarray(x, dtype=np.float32)
    xh = (
        x.reshape(B, T, H, HD).transpose(0, 2, 1, 3).reshape(PAIRS, T, HD)
    )
    in_maps = [
        {"x": np.ascontiguousarray(xh[i * PPC : (i + 1) * PPC])}
        for i in range(NCORES)
    ]
    for _attempt in range(3):
        res = run_bass_kernel_spmd(nc, in_maps, core_ids=list(range(NCORES)))
        outs = np.concatenate(
            [np.asarray(res.results[i]["out"]) for i in range(NCORES)], axis=0
        )
        if np.isfinite(outs).all():
            break
    return (
        outs.reshape(B, H, T, HD).transpose(0, 2, 1, 3).reshape(B, T, D)
    ).astype(np.float32)



# revision 72
# speedup vs baseline: 1.1395x; 1.0029x over previous
"""AutoCorrelation kernel for Trainium2, 8 NeuronCores.

Math per (b, h) pair with X = x[b, :, h*64:(h+1)*64]  [T=2048, hd=64]:
  Xc = X - mean_T(X)
  S  = Xc @ Xc.T                  (symmetric!)
  P  = softmax(S, axis=-1)
  out = P @ X

Implementation exploits symmetry of E = exp(S - 64):
  out[t] = (sum_T' E[t,T'] X[T']) / (sum_T' E[t,T'])
and E == E.T, so the row-blocks of E computed with t on partitions can be
used directly as the *streaming* operand of the PV matmul (lhsT = [X | 1]),
which also yields the softmax denominator L in output row 64. No transposes
of the attention matrix are ever needed.

Engine plan per core (8 independent (b,h) pairs, data parallel across 8
cores): S-matmuls run 2x via PE row-tiling (K=64 on tiles T0/T8); exp is
split between ScalarE (table exp) and VectorE (Schraudolph bf16 bit-trick
with saturating f32->u16 convert); the PV matmuls for the previous pair are
spread between S blocks so ScalarE never starves; prep (DMA, paired
transposes, centering) for the next pair is interleaved too.
"""

import numpy as np

NCORES = 8
B, T, D, H = 4, 2048, 1024, 16
HD = D // H            # 64
PAIRS = B * H          # 64
PPC = PAIRS // NCORES  # 8 pairs per core
KT = T // 128          # 16 row-blocks of 128

# exp split between ScalarE (table exp, ~(N+352)/1.2 ns) and VectorE
# (Schraudolph bf16 bit-trick, f32 PSUM in -> 1x, ~(N+130)/0.96 ns).
# Each S block yields 4 single-bank [128,512] PSUM units; units j=(0,1)
# and (2,3) come from concurrent row-tiled matmul pairs, so their exp
# engines MUST alternate (s,v) on every adjacent slot pair -> both banks
# free together and ALL next-block matmul pairs stay concurrent. The
# forced 1:1 exp split is rebalanced by moving the copy-aux (vb/xb/sbt)
# to ScalarE.
UNIT_ENGINE = (("s", "v", "s", "v"), ("s", "v", "s", "v"))
SCHRAUD_A = 128.0 / float(np.log(2.0))               # 184.6649...
SCHRAUD_B = 127.0 * 128.0 - 5.25 - 64.0 * SCHRAUD_A  # bf16 bits bias, folds exp(-64)

_CACHE = {}


def _build_nc():
    import concourse.bass as bass  # noqa: F401
    import concourse.tile as tile
    from concourse import bacc, mybir
    from concourse.masks import make_identity

    f32 = mybir.dt.float32
    f16 = mybir.dt.float16
    bf16 = mybir.dt.bfloat16
    u16 = mybir.dt.uint16
    ADD = mybir.AluOpType.add
    MULT = mybir.AluOpType.mult
    EXP = mybir.ActivationFunctionType.Exp

    nc = bacc.Bacc(None)
    x_ext = nc.declare_dram_parameter("x", [PPC, T, HD], f32, isOutput=False)
    o_ext = nc.declare_dram_parameter("out", [PPC, T, HD], f32, isOutput=True)

    x_t = x_ext.ap().rearrange("p (ko pp) d -> p pp ko d", pp=128)
    o_t = o_ext.ap().rearrange("p (ko pp) d -> p pp ko d", pp=128)

    with tile.TileContext(nc) as tc:
        with (
            tc.tile_pool(name="const", bufs=1) as constp,
            tc.tile_pool(name="xst", bufs=2) as xstp,
            tc.tile_pool(name="stage", bufs=2) as stagep,
            tc.tile_pool(name="xct", bufs=2) as xctp,
            tc.tile_pool(name="vb", bufs=2) as vbp,
            tc.tile_pool(name="eb", bufs=2) as ebp,
            tc.tile_pool(name="osb", bufs=2) as osbp,
            tc.tile_pool(name="sbt", bufs=2) as sbtp,
            tc.tile_pool(name="small", bufs=4) as smallp,
            tc.tile_pool(name="psS", bufs=3, space="PSUM") as psSp,
            tc.tile_pool(name="psM", bufs=2, space="PSUM") as psMp,
        ):
            neg64 = constp.tile([128, 1], f32)
            nc.vector.memset(neg64, -64.0)
            ident = constp.tile([128, 128], bf16)
            make_identity(nc, ident)
            identf = constp.tile([HD + 1, HD + 1], f32)
            make_identity(nc, identf)

            state = {}
            xst_tiles = {}

            def emit_dma_in(p):
                # split across two DMA queues: the 256B-element gather is
                # descriptor-rate-bound (~5.7us on one queue)
                xst = xstp.tile([128, KT, HD], f32, tag="xst")
                nc.gpsimd.dma_start(xst[:, 0:8], x_t[p][:, 0:8])
                nc.sync.dma_start(xst[:, 8:16], x_t[p][:, 8:16])
                xst_tiles[p] = xst

            def emit_prep_a(p):
                # V (with ones column) + packed bf16 copy for DMA-transposes
                xst = xst_tiles[p]
                vb = vbp.tile([128, KT, HD + 1], bf16, tag="vb")
                nc.vector.memset(vb[:, :, HD : HD + 1], 1.0)
                nc.scalar.activation(
                    vb[:, :, 0:HD], xst,
                    mybir.ActivationFunctionType.Copy,
                )
                xb = stagep.tile([128, KT * HD], bf16, tag="xb")
                nc.scalar.activation(
                    xb.rearrange("p (k d) -> p k d", d=HD), xst,
                    mybir.ActivationFunctionType.Copy,
                )
                xct = xctp.tile([128, T], bf16, tag="xct")
                E = ebp.tile([128, KT, T], bf16, tag="eb")
                state[p] = {"E": E, "vb": vb, "xct": xct, "xb": xb}
                if p > 0:
                    state[p]["stage"] = stagep.tile(
                        [128, 2 * 512], bf16, tag="stage", name="stage"
                    )
                xst_tiles.pop(p)

            def emit_prep_b(p):
                # XT: out[p, b, f] = in[f, b*128+p] (partitions 0:64 get
                # even k-tile's d, 64:128 odd k-tile's d). Steady state uses
                # the HW DMA-transpose; pair 0 uses PE transposes instead
                # (PE idle at startup, and the DMA xbar has ~9us latency
                # that would sit on the startup critical path).
                xb = state[p]["xb"]
                if p > 0:
                    nc.sync.dma_start_transpose(
                        state[p]["stage"].rearrange("p (q f) -> p q f", f=128),
                        xb,
                    )
                    return
                xbv = xb.rearrange("p (k d) -> p k d", d=HD)
                psT = [
                    psMp.tile([HD, 1024], bf16, tag="mix", name=f"psT{h}")
                    for h in range(2)
                ]
                for k in range(KT):
                    nc.tensor.transpose(
                        psT[k % 2][:, (k // 2) * 128 : (k // 2 + 1) * 128],
                        xbv[:, k, :], ident,
                    )
                state[p]["psT"] = psT

            def emit_prep_bc0():
                # pair-0-only fast path: mean + centering straight from the
                # PE-transposed PSUM tiles into xct's upper half (engines
                # cannot cross partitions), then one big-descriptor DMA
                # duplicates partitions 0:64 -> 64:128.
                psT = state[0].pop("psT")
                xct = state[0]["xct"]
                xg = xct.rearrange("p (k f) -> p k f", f=128)
                part0 = smallp.tile([HD, 1], f32, tag="part")
                nc.vector.tensor_reduce(
                    part0, psT[0], mybir.AxisListType.X, ADD
                )
                scr = sbtp.tile([HD, 1024], bf16, tag="scr0", name="scr0")
                part1 = smallp.tile([HD, 1], f32, tag="swap", name="part1")
                nc.scalar.activation(
                    scr, psT[1], mybir.ActivationFunctionType.Identity,
                    accum_out=part1,
                )
                # the rest of the chain avoids the vector queue: the
                # scheduler likes hoisting pair-1 prep ahead of these ops,
                # which would head-of-line block them for ~7us
                mu64 = smallp.tile([HD, 1], f32, tag="mufull", name="mu64")
                nc.gpsimd.tensor_tensor(mu64, part0, part1, ADD)
                nc.gpsimd.tensor_scalar_mul(mu64, mu64, -1.0 / T)
                for h in range(2):
                    nc.scalar.activation(
                        xg[0:HD, h : KT : 2, :],
                        psT[h].rearrange("p (k f) -> p k f", f=128),
                        mybir.ActivationFunctionType.Identity,
                        bias=mu64,
                    )
                # centered upper half -> duplicate to partitions 64:128
                nc.sync.dma_start(xct[HD:128, :], xct[0:HD, :])

            def emit_prep_b2(p):
                # mean over T from the transposed stage (free-axis reduce).
                # Partitions 0:64 and 64:128 hold the even/odd k-tile
                # partial sums of the same d; a half-swap stream_shuffle +
                # add gives the full sum in BOTH halves at once (symmetric),
                # so no cross-partition DMA or broadcast is needed.
                stage = state[p]["stage"]
                part = smallp.tile([128, 1], f32, tag="part")
                nc.vector.tensor_reduce(
                    part, stage.rearrange("p (q f) -> p q f", f=128),
                    mybir.AxisListType.XY, ADD,
                )
                swap = smallp.tile([128, 1], f32, tag="swap")
                nc.vector.stream_shuffle(
                    swap, part, [(g + 16) % 32 for g in range(32)]
                )
                mufull = smallp.tile([128, 1], f32, tag="mufull")
                nc.vector.tensor_tensor(part, part, swap, ADD)
                nc.vector.tensor_scalar_mul(mufull, part, -1.0 / T)
                if p == 0:
                    # startup: keep the centering off the vector queue so
                    # pair-1 prep scheduled ahead of it cannot stall it
                    nc.scalar.activation(
                        stage, stage,
                        mybir.ActivationFunctionType.Identity, bias=mufull,
                    )
                else:
                    nc.vector.tensor_scalar(stage, stage, mufull, None, ADD)
                state[p]["mufull"] = mufull

            def emit_prep_c(p):
                # shuffle stage -> xct (both halves get all 16 k-tiles);
                # pair 0's shuffles are on the startup critical path, so
                # half go out on the sync queue in parallel
                stage = state[p]["stage"]
                xct = state[p]["xct"]
                sg = stage.rearrange("p (q f) -> p q f", f=128)
                xg = xct.rearrange("p (k f) -> p k f", f=128)
                eng = nc.sync if p == 0 else nc.gpsimd
                nc.gpsimd.dma_start(xg[0:HD, 0:KT:2, :], sg[0:HD])
                eng.dma_start(xg[0:HD, 1:KT:2, :], sg[HD:128])
                nc.gpsimd.dma_start(xg[HD:128, 0:KT:2, :], sg[0:HD])
                eng.dma_start(xg[HD:128, 1:KT:2, :], sg[HD:128])

            def emit_s_exp(p, m):
                E = state[p]["E"]
                xct = state[p]["xct"]
                ms = slice(m * 128, (m + 1) * 128)
                # two [128,1024] pair-tiles; pair n holds BOTH outputs of
                # the concurrent row-tiled matmul pair (T0 -> cols 0:512,
                # T8 -> cols 512:1024, each within one bank). One exp op
                # consumes a whole pair-tile (strided E dst), so the two
                # banks of the next pair free ATOMICALLY -> the matmul
                # pair is always concurrent.
                Ev = E[:, m, :].rearrange("p (h f) -> p h f", f=1024)
                for n in range(2):
                    ps = psSp.tile(
                        [128, 1024], f32, tag="psS", name=f"psS{n}"
                    )
                    nc.tensor.matmul(
                        ps[:, 0:512],
                        lhsT=xct[0:HD, ms],
                        rhs=xct[0:HD, n * 512 : (n + 1) * 512],
                        start=True, stop=True, tile_position=(0, 0),
                    )
                    nc.tensor.matmul(
                        ps[:, 512:1024],
                        lhsT=xct[HD:128, ms],
                        rhs=xct[HD:128, 1024 + n * 512 : 1024 + (n + 1) * 512],
                        start=True, stop=True, tile_position=(64, 0),
                    )
                    eview = Ev[:, :, n * 512 : (n + 1) * 512]
                    psv = ps.rearrange("p (h f) -> p h f", f=512)
                    if n == 1:
                        # Schraudolph in bf16 bit-space; f32->u16 convert
                        # saturates negatives to 0 (== exp underflow).
                        nc.vector.tensor_scalar(
                            eview.bitcast(u16), psv, SCHRAUD_A, SCHRAUD_B,
                            MULT, ADD,
                        )
                    else:
                        nc.scalar.activation(
                            eview, psv, EXP, bias=neg64, scale=1.0
                        )

            pv_live = {}

            def emit_pv_part(q, c, part):
                # 8 of the 16 accumulating PV matmuls for chunk c of pair q
                E, vb = state[q]["E"], state[q]["vb"]
                cs = slice(c * 512, (c + 1) * 512)
                if part == 0:
                    pv_live[(q, c)] = psMp.tile(
                        [HD + 1, 512], f32, tag="mix", name="pspv"
                    )
                pspv = pv_live[(q, c)]
                for kk in range(8):
                    k = part * 8 + kk
                    nc.tensor.matmul(
                        pspv,
                        lhsT=vb[:, k, :],
                        rhs=E[:, k, cs],
                        start=(k == 0), stop=(k == KT - 1),
                        skip_group_check=True,
                    )

            sbts = {}

            def emit_pv_tail(q, c):
                # PSUM -> bf16 SBUF copy per chunk; frees the PV PSUM slot.
                pspv = pv_live.pop((q, c))
                if q == PPC - 1:
                    # last pair: per-chunk PE transposes (PE idles in the
                    # final phase and the xbar's ~5us dispatch latency would
                    # serialize the drain).
                    sbt = sbtp.tile(
                        [HD + 1, 512], f32, tag="sbtL", name="sbtL"
                    )
                    nc.vector.tensor_copy(sbt, pspv)
                    ps2 = psMp.tile(
                        [128, 4, HD + 1], f32, tag="mix", name="ps2L"
                    )
                    for j in range(4):
                        nc.tensor.transpose(
                            ps2[:, j, :],
                            sbt[:, j * 128 : (j + 1) * 128],
                            identf,
                        )
                    lrec = smallp.tile([128, 4], f32, tag="lrecL", name="lrecL")
                    nc.vector.reciprocal(lrec, ps2[:, :, HD])
                    osbc = osbp.tile([128, 4, HD], f32, tag="osbL", name="osbL")
                    nc.vector.tensor_tensor(
                        osbc, ps2[:, :, 0:HD],
                        lrec[:, :, None].to_broadcast([128, 4, HD]), MULT,
                    )
                    eng = nc.gpsimd if c % 2 == 0 else nc.sync
                    eng.dma_start(o_t[q][:, c * 4 : (c + 1) * 4, :], osbc)
                    return
                if c == 0:
                    sbts[q] = sbtp.tile([80, T], bf16, tag="sbt", name="sbt")
                nc.scalar.activation(
                    sbts[q][0 : HD + 1, c * 512 : (c + 1) * 512], pspv,
                    mybir.ActivationFunctionType.Copy,
                )


            out_live = {}

            def emit_pair_out_a(q):
                # ONE whole-pair HW DMA-transpose (xbar, 16x128 tiles; 80 =
                # padded partition count) gives the [t, d] layout without
                # touching the Tensor engine. Emitted an iteration after the
                # last sbt copy so its wait never head-of-line blocks the
                # next pair's prep transpose on the sync queue. Rows 65:80
                # of sbt are never written and land at f=65:80 of ps2t,
                # outside every consumed slice.
                sbt = sbts.pop(q)
                ps2t = sbtp.tile([128, KT, 80], bf16, tag="ps2t")
                nc.sync.dma_start_transpose(ps2t, sbt)
                out_live[q] = ps2t

            def emit_pair_out_b(q):
                # emitted ~5us after the dispatch so the xbar's transfer
                # latency never stalls the vector queue head
                ps2t = out_live.pop(q)
                lrec = smallp.tile([128, KT], f32, tag="lrec")
                nc.vector.reciprocal(lrec, ps2t[:, :, HD])
                osb = osbp.tile([128, KT, HD], f32, tag="osb")
                nc.vector.tensor_tensor(
                    osb, ps2t[:, :, 0:HD],
                    lrec[:, :, None].to_broadcast([128, KT, HD]), MULT,
                )
                nc.gpsimd.dma_start(o_t[q][:, 0:8], osb[:, 0:8])
                nc.sync.dma_start(o_t[q][:, 8:16], osb[:, 8:16])

            emit_dma_in(0)
            emit_prep_a(0)
            emit_prep_b(0)
            emit_prep_bc0()
            emit_dma_in(1)

            for it in range(PPC + 1):
                for m in range(KT):
                    if it < PPC:
                        emit_s_exp(it, m)
                    if it > 0 and m % 2 == 1:
                        emit_pv_part(it - 1, m // 4, (m % 4) // 2)
                        if m % 4 == 3 and it < PPC:
                            emit_pv_tail(it - 1, m // 4)
                        # last pair: delay each tail 2 slots so its PE
                        # transposes never head-of-line block the next
                        # PV group behind the sbt copy
                        if it == PPC and m % 4 == 1 and m > 4:
                            emit_pv_tail(it - 1, (m - 5) // 4)
                    if m == 5 and it >= 2 and it - 2 < PPC - 1:
                        emit_pair_out_a(it - 2)
                    if m == 1 and it >= 3 and it - 3 < PPC - 1:
                        emit_pair_out_b(it - 3)
                    if it + 1 < PPC:
                        if m == 2:
                            emit_prep_a(it + 1)
                        elif m == 3:
                            emit_prep_b(it + 1)
                        elif m == 6:
                            emit_prep_b2(it + 1)
                        elif m == 7:
                            emit_prep_c(it + 1)
                        elif m == 9 and it + 2 < PPC:
                            emit_dma_in(it + 2)
                if it > 0:
                    state.pop(it - 1)
            emit_pv_tail(PPC - 1, 3)
            emit_pair_out_b(PPC - 2)
    nc.compile()
    return nc


def _get_nc():
    if "nc" not in _CACHE:
        _CACHE["nc"] = _build_nc()
    return _CACHE["nc"]


def kernel(x: np.ndarray) -> np.ndarray:
    from concourse.bass_utils import run_bass_kernel_spmd

    nc = _get_nc()
    x = np.asarray(x, dtype=np.float32)
    xh = (
        x.reshape(B, T, H, HD).transpose(0, 2, 1, 3).reshape(PAIRS, T, HD)
    )
    in_maps = [
        {"x": np.ascontiguousarray(xh[i * PPC : (i + 1) * PPC])}
        for i in range(NCORES)
    ]
    for _attempt in range(3):
        res = run_bass_kernel_spmd(nc, in_maps, core_ids=list(range(NCORES)))
        outs = np.concatenate(
            [np.asarray(res.results[i]["out"]) for i in range(NCORES)], axis=0
        )
        if np.isfinite(outs).all():
            break
    return (
        outs.reshape(B, H, T, HD).transpose(0, 2, 1, 3).reshape(B, T, D)
    ).astype(np.float32)

